# revision 5
# baseline (speedup 1.0000x reference)
"""
Trainium2 Bass kernel v2 for nn_ARqGPS (autoregressive qGPS log-amplitude).

Math:
  H[b,m,t]  = prod_{j<=t} eps_sel[b,m,j],  eps_sel = eps[x[b,t],m,t]
  rp[b,t]   = sum_m H[b,m,t]
  rs[b,t]   = sum_m (eps0+eps1)[m,t] * H[b,m,t-1]   (H[.,.,-1] = 1)
  ro = rs - rp;  term = notmask * (rp - mx - 0.5*log1p(exp(2*(mn-mx))))
  out[b] = sum_t term[b,t]

Device (per core, 128 batch rows, t on partitions in 2 chunks of 128):
  T1a[t,(b,m)] = x*lr          (DVE f16 2x via x-replicated-pair broadcast)
  S = TRI@T1a + I@C0_bc (+ ONES@T1a_c0 for chunk1)   (PE, f16 rhs, f32 psum)
      C0[t,m] = cumsum_t log eps0  -- host precomputed
  H = exp(S)                   (ACT, 1024-free reads from 2-bank psum tiles)
  WH = H*wsh_bc                (DVE; gpsimd bulk ops poison DVE SBUF access)
  rp/rsn = binary-tree m-reduction in f16 (fp32 final)        (DVE)
  tail: counts via STRI matmul, t-1 shift via SHM/SH2 matmuls, exp/ln
  softplus, masked sum via ones-column matmul.
"""
import os
import sys

import numpy as np

for _p in ("/opt/trn_rl_repo", os.path.expanduser("~/.axon_site/_ro/trn_rl_repo")):
    if os.path.isdir(_p) and _p not in sys.path:
        sys.path.insert(0, _p)
        break

import concourse.bass as bass
import concourse.bacc as bacc
import concourse.mybir as mybir
from concourse.tile import TileContext

B, L, M = 1024, 256, 128
NCORES = 8
BLOC = B // NCORES          # 128 batch rows per core
HALF = L // 2
WB = 16                     # batch rows per wide block
NWB = BLOC // WB            # 8 wide blocks
FB = WB * M                 # 2048 free elems per chunk per wblock
NCHUNK = 2

F32 = mybir.dt.float32
F16 = mybir.dt.float16
AF = mybir.ActivationFunctionType
ALU = mybir.AluOpType
AX = mybir.AxisListType


def build_nc():
    nc = bacc.Bacc("TRN2", target_bir_lowering=False)
    xr2 = nc.dram_tensor("xr2", (128, NCHUNK * WB * NWB * 2), F16,
                         kind="ExternalInput")          # (t, c, b, 2)
    lrt = nc.dram_tensor("lrt", (128, NCHUNK * M), F16, kind="ExternalInput")
    c0t = nc.dram_tensor("c0t", (128, NCHUNK * M), F16, kind="ExternalInput")
    wsht = nc.dram_tensor("wsht", (128, NCHUNK * M), F16, kind="ExternalInput")
    tri3 = nc.dram_tensor("tri3", (128, 3 * 128), F16, kind="ExternalInput")
    shms = nc.dram_tensor("shms", (128, 2 * 128), F32, kind="ExternalInput")
    tvc = nc.dram_tensor("tvc", (128, 4), F32, kind="ExternalInput")
    one0 = nc.dram_tensor("one0", (1, BLOC), F32, kind="ExternalInput")
    cst0 = nc.dram_tensor("cst0", (1, BLOC), F32, kind="ExternalInput")
    y = nc.dram_tensor("y", (1, BLOC), F32, kind="ExternalOutput")

    with TileContext(nc) as tc:
        with (
            tc.tile_pool(name="const", bufs=1) as cpool,
            tc.tile_pool(name="t1p", bufs=3) as t1pool,
            tc.tile_pool(name="hp", bufs=3) as hpool,
            tc.tile_pool(name="whp", bufs=3) as whpool,
            tc.tile_pool(name="s1p", bufs=3) as s1pool,
            tc.tile_pool(name="red", bufs=1) as rpool,
            tc.tile_pool(name="small", bufs=1) as spool,
            tc.tile_pool(name="ps", bufs=2, space="PSUM") as pspool,
            tc.tile_pool(name="psm", bufs=1, space="PSUM") as psmisc,
        ):
            # ---- constants ----
            XR2 = cpool.tile([128, NCHUNK * BLOC * 2], F16, tag="XR2")
            LRT = cpool.tile([128, NCHUNK * M], F16, tag="LRT")
            C0T = cpool.tile([128, NCHUNK * M], F16, tag="C0T")
            WSHT = cpool.tile([128, NCHUNK * M], F16, tag="WSHT")
            TRI3 = cpool.tile([128, 3 * 128], F16, tag="TRI3")
            SHMS = cpool.tile([128, 2 * 128], F32, tag="SHMS")
            TVC = cpool.tile([128, 4], F32, tag="TVC")
            ONE0 = cpool.tile([1, BLOC], F32, tag="ONE0")
            CST0 = cpool.tile([1, BLOC], F32, tag="CST0")
            # critical-path first: first wblocks need xr2 q0 (c0 b<64) and
            # q2 (c1 b<64), then TRI3 for the first matmuls
            nc.sync.dma_start(LRT[:], lrt[:])
            nc.sync.dma_start(XR2[:, 0:32], xr2[:, 0:32])
            nc.sync.dma_start(XR2[:, 256:288], xr2[:, 256:288])
            nc.sync.dma_start(TRI3[:], tri3[:])
            nc.sync.dma_start(XR2[:, 32:128], xr2[:, 32:128])
            nc.sync.dma_start(XR2[:, 288:384], xr2[:, 288:384])
            nc.sync.dma_start(C0T[:], c0t[:])
            for dq in (1, 3):
                qs = slice(dq * 128, (dq + 1) * 128)
                nc.sync.dma_start(XR2[:, qs], xr2[:, qs])
            nc.sync.dma_start(WSHT[:], wsht[:])
            nc.sync.dma_start(TVC[:], tvc[:])
            nc.sync.dma_start(ONE0[:], one0[:])
            nc.sync.dma_start(CST0[:], cst0[:])
            nc.sync.dma_start(SHMS[:], shms[:])
            TRI = TRI3[:, 0:128]
            STRI = TRI3[:, 128:256]
            IDT = TRI3[:, 256:384]
            SHM = SHMS[:, 0:128]
            SH2 = SHMS[:, 128:256]
            ONES16 = cpool.tile([128, 128], F16, tag="ONES16")
            nc.vector.memset(ONES16[:], 1.0)
            ONEC = cpool.tile([128, 1], F32, tag="ONEC")
            nc.vector.memset(ONEC[:], 1.0)
            # (t, c, b, 2) view of XR2
            XRV = XR2[:].rearrange("p (c b two) -> p c b two", c=NCHUNK, two=2)

            # ---- persistent reduction tensors ----
            R64 = rpool.tile([128, NCHUNK, BLOC, 64], F16, tag="R64")
            W64 = rpool.tile([128, NCHUNK, BLOC, 64], F16, tag="W64")
            RP = spool.tile([128, NCHUNK, BLOC], F32, tag="RP")
            RSN = spool.tile([128, NCHUNK, BLOC], F32, tag="RSN")

            # ---- main loop ----
            for wb in range(NWB):
                bsl = slice(wb * WB, (wb + 1) * WB)
                t1 = []
                for c in range(NCHUNK):
                    t = t1pool.tile([128, WB, 64, 2], F16, tag=f"T1_{c}")
                    xbc = (XRV[:, c, bsl, :].unsqueeze(2)
                           .broadcast_to([128, WB, 64, 2]))
                    lbc = (LRT[:, c * M:(c + 1) * M]
                           .rearrange("p (a two) -> p a two", two=2)
                           .unsqueeze(1).broadcast_to([128, WB, 64, 2]))
                    nc.vector.tensor_tensor(t[:], xbc, lbc, ALU.mult)
                    t1.append(t)
                ht = []
                for c in range(NCHUNK):
                    h = hpool.tile([128, WB, M], F16, tag=f"H_{c}")
                    t1f = t1[c][:].rearrange("p b a two -> p (b a two)")
                    t1f0 = t1[0][:].rearrange("p b a two -> p (b a two)")
                    c0bc = (C0T[:, c * M:(c + 1) * M].unsqueeze(1)
                            .broadcast_to([128, 4, M]))
                    P0 = pspool.tile([128, 1024], F32, tag="S")
                    P1 = pspool.tile([128, 1024], F32, tag="S")
                    P = [P0, P1]
                    # 512-wide matmuls (psum bank limit), grouped by weights
                    for q in range(2):
                        for hh in range(2):
                            sl = slice((2 * q + hh) * 512,
                                       (2 * q + hh + 1) * 512)
                            nc.tensor.matmul(P[q][:, hh * 512:(hh + 1) * 512],
                                             TRI, t1f[:, sl],
                                             start=True, stop=False)
                    for q in range(2):
                        for hh in range(2):
                            po = P[q][:, hh * 512:(hh + 1) * 512]
                            nc.tensor.matmul(po, IDT, c0bc,
                                             start=False, stop=(c == 0))
                    if c == 1:
                        for q in range(2):
                            for hh in range(2):
                                sl = slice((2 * q + hh) * 512,
                                           (2 * q + hh + 1) * 512)
                                nc.tensor.matmul(
                                    P[q][:, hh * 512:(hh + 1) * 512],
                                    ONES16, t1f0[:, sl],
                                    start=False, stop=True)
                    for q in range(2):
                        hv = (h[:].rearrange("p b m -> p (b m)")
                              [:, q * 1024:(q + 1) * 1024])
                        nc.scalar.activation(hv, P[q][:], AF.Exp)
                    ht.append(h)
                for c in range(NCHUNK):
                    w = whpool.tile([128, WB, M], F16, tag=f"W_{c}")
                    wbc = (WSHT[:, c * M:(c + 1) * M].unsqueeze(1)
                           .broadcast_to([128, WB, M]))
                    nc.vector.tensor_tensor(w[:], ht[c][:], wbc, ALU.mult)
                    # tree step 1 straight into the persistent pyramids
                    nc.vector.tensor_tensor(R64[:, c, bsl, :],
                                            ht[c][:, :, 0:64],
                                            ht[c][:, :, 64:128], ALU.add)
                    nc.vector.tensor_tensor(W64[:, c, bsl, :],
                                            w[:, :, 0:64],
                                            w[:, :, 64:128], ALU.add)

            # ---- global tree steps + final reduce ----
            R32 = rpool.tile([128, NCHUNK, BLOC, 32], F16, tag="R32")
            R16 = rpool.tile([128, NCHUNK, BLOC, 16], F16, tag="R16")
            R8 = rpool.tile([128, NCHUNK, BLOC, 8], F16, tag="R8")
            for srcp, dst in ((R64, RP), (W64, RSN)):
                nc.vector.tensor_tensor(R32[:], srcp[:, :, :, 0:32],
                                        srcp[:, :, :, 32:64], ALU.add)
                nc.vector.tensor_tensor(R16[:], R32[:, :, :, 0:16],
                                        R32[:, :, :, 16:32], ALU.add)
                nc.vector.tensor_tensor(R8[:], R16[:, :, :, 0:8],
                                        R16[:, :, :, 8:16], ALU.add)
                nc.vector.tensor_reduce(dst[:], R8[:], AX.X, ALU.add)

            # ---- tail ----
            # exclusive spin-up counts c1[t,b] via strict-lower-tri matmuls
            X0 = XRV[:, 0, :, 0]                  # (128, BLOC) stride-2 view
            X1 = XRV[:, 1, :, 0]
            C1p = psmisc.tile([128, NCHUNK, BLOC], F32, tag="C1p")
            nc.tensor.matmul(C1p[:, 0, :], STRI, X0, start=True, stop=True)
            nc.tensor.matmul(C1p[:, 1, :], STRI, X1, start=True, stop=False)
            nc.tensor.matmul(C1p[:, 1, :], ONES16, X0, start=False, stop=True)
            # r_sum aligned: RSA[t] = RSN[t-1]; RSA[0] = s0 const
            RSA = psmisc.tile([128, NCHUNK, BLOC], F32, tag="RSA")
            nc.tensor.matmul(RSA[:, 0, :], SHM, RSN[:, 0, :],
                             start=True, stop=False)
            nc.tensor.matmul(RSA[:, 0, :], ONE0[:], CST0[:],
                             start=False, stop=True)
            nc.tensor.matmul(RSA[:, 1, :], SHM, RSN[:, 1, :],
                             start=True, stop=False)
            nc.tensor.matmul(RSA[:, 1, :], SH2, RSN[:, 0, :],
                             start=False, stop=True)
            # n_other = c1 + x*(t - 2*c1); notmask = n_other < HALF
            NM = spool.tile([128, NCHUNK, BLOC], F32, tag="NM")
            UT = spool.tile([128, NCHUNK, BLOC], F32, tag="UT")
            for c in range(NCHUNK):
                xc = XRV[:, c, :, 0]
                nc.vector.tensor_scalar(UT[:, c, :], C1p[:, c, :], -2.0,
                                        TVC[:, c:c + 1], ALU.mult, ALU.add)
                nc.vector.tensor_tensor(UT[:, c, :], UT[:, c, :], xc, ALU.mult)
                nc.vector.tensor_tensor(UT[:, c, :], UT[:, c, :], C1p[:, c, :],
                                        ALU.add)
                nc.vector.tensor_single_scalar(NM[:, c, :], UT[:, c, :],
                                               float(HALF) - 0.5, ALU.is_lt)
            # term = notmask * (rp - mx - 0.5*log1p(exp(2*(mn-mx))))
            RO = spool.tile([128, NCHUNK, BLOC], F32, tag="RO")
            MX = spool.tile([128, NCHUNK, BLOC], F32, tag="MX")
            MN = spool.tile([128, NCHUNK, BLOC], F32, tag="MN")
            SPt = spool.tile([128, NCHUNK, BLOC], F32, tag="SPt")
            TERM = spool.tile([128, NCHUNK, BLOC], F32, tag="TERM")
            nc.vector.tensor_tensor(RO[:], RSA[:], RP[:], ALU.subtract)
            nc.vector.tensor_tensor(MX[:], RP[:], RO[:], ALU.max)
            nc.vector.tensor_tensor(MN[:], RP[:], RO[:], ALU.min)
            nc.vector.tensor_tensor(MN[:], MN[:], MX[:], ALU.subtract)
            nc.scalar.activation(SPt[:], MN[:], AF.Exp, scale=2.0)
            nc.scalar.activation(SPt[:], SPt[:], AF.Ln, bias=1.0)
            nc.vector.tensor_tensor(MX[:], RP[:], MX[:], ALU.subtract)
            nc.vector.scalar_tensor_tensor(TERM[:], SPt[:], -0.5, MX[:],
                                           ALU.mult, ALU.add)
            nc.vector.tensor_tensor(TERM[:], TERM[:], NM[:], ALU.mult)
            # out[b] = sum_t term
            YPp = psmisc.tile([1, NCHUNK * BLOC], F32, tag="YPp")
            nc.tensor.matmul(YPp[:], ONEC[:],
                             TERM[:].rearrange("p a b -> p (a b)"),
                             start=True, stop=True)
            YS = spool.tile([1, NCHUNK * BLOC], F32, tag="YS")
            nc.scalar.activation(YS[:], YPp[:], AF.Copy)
            YF = spool.tile([1, BLOC], F32, tag="YF")
            nc.vector.tensor_tensor(YF[:], YS[0:1, 0:BLOC],
                                    YS[0:1, BLOC:2 * BLOC], ALU.add)
            nc.sync.dma_start(y[:], YF[:])
    nc.compile()
    return nc


def host_tables(inputs, epsilon):
    x = np.asarray(inputs).astype(np.float32)        # (B, L)
    eps = np.asarray(epsilon).astype(np.float64)     # (2, M, L)
    eps0, eps1 = eps[0], eps[1]
    le0 = np.log(eps0)                               # (M, L)
    le1 = np.log(eps1)
    lr = (le1 - le0)                                 # (M, L)
    c0 = np.cumsum(le0, axis=1)                      # (M, L)
    w = eps0 + eps1
    wsh = np.zeros((M, L))
    wsh[:, :L - 1] = w[:, 1:]
    s0 = np.float32(w[:, 0].sum())

    def chunked_t(a):   # (M, L) -> (128, 2*M): [:, c*M:(c+1)*M] = a[:, c-chunk].T
        return np.concatenate([a[:, c * 128:(c + 1) * 128].T
                               for c in range(NCHUNK)], axis=1)

    lrt = chunked_t(lr).astype(np.float16)
    c0t = chunked_t(c0).astype(np.float16)
    wsht = chunked_t(wsh).astype(np.float16)

    ar = np.arange(128)
    tri = (ar[:, None] <= ar[None, :]).astype(np.float16)
    stri = (ar[:, None] < ar[None, :]).astype(np.float16)
    idt = (ar[:, None] == ar[None, :]).astype(np.float16)
    tri3 = np.ascontiguousarray(np.concatenate([tri, stri, idt], axis=1))
    shm = (ar[:, None] == (ar[None, :] - 1)).astype(np.float32)
    sh2 = ((ar[:, None] == 127) & (ar[None, :] == 0)).astype(np.float32)
    shms = np.ascontiguousarray(np.concatenate([shm, sh2], axis=1))
    tvc = np.zeros((128, 4), np.float32)
    tvc[:, 0] = ar
    tvc[:, 1] = ar + 128.0

    tables = {
        "lrt": lrt, "c0t": c0t, "wsht": wsht, "tri3": tri3, "shms": shms,
        "tvc": tvc,
        "one0": (np.arange(BLOC)[None, :] == 0).astype(np.float32),
        "cst0": np.full((1, BLOC), s0, np.float32),
    }
    # xr2 per core built later: (128, c, b, 2) f16
    xt = x.T                                         # (L, B)
    return tables, xt


_NC_CACHE = {}


def get_nc():
    if "nc" not in _NC_CACHE:
        _NC_CACHE["nc"] = build_nc()
    return _NC_CACHE["nc"]


def make_xr2(xt, k):
    xc = xt[:, k * BLOC:(k + 1) * BLOC]              # (L, BLOC)
    xr2 = np.empty((128, NCHUNK, BLOC, 2), np.float16)
    for c in range(NCHUNK):
        xr2[:, c, :, 0] = xc[c * 128:(c + 1) * 128]
        xr2[:, c, :, 1] = xc[c * 128:(c + 1) * 128]
    return np.ascontiguousarray(xr2.reshape(128, -1))


def kernel(inputs, epsilon):
    from concourse.bass_utils import run_bass_kernel_spmd

    tables, xt = host_tables(inputs, epsilon)
    nc = get_nc()
    in_maps = []
    for k in range(NCORES):
        m = dict(tables)
        m["xr2"] = make_xr2(xt, k)
        in_maps.append(m)
    res = run_bass_kernel_spmd(nc, in_maps, core_ids=list(range(NCORES)))
    out = np.empty((B,), np.float32)
    for k in range(NCORES):
        out[k * BLOC:(k + 1) * BLOC] = np.asarray(res.results[k]["y"]).reshape(-1)
    return out


# revision 7
# speedup vs baseline: 1.0353x; 1.0353x over previous
"""
Trainium2 Bass kernel v2 for nn_ARqGPS (autoregressive qGPS log-amplitude).

Math:
  H[b,m,t]  = prod_{j<=t} eps_sel[b,m,j],  eps_sel = eps[x[b,t],m,t]
  rp[b,t]   = sum_m H[b,m,t]
  rs[b,t]   = sum_m (eps0+eps1)[m,t] * H[b,m,t-1]   (H[.,.,-1] = 1)
  ro = rs - rp;  term = notmask * (rp - mx - 0.5*log1p(exp(2*(mn-mx))))
  out[b] = sum_t term[b,t]

Device (per core, 128 batch rows, t on partitions in 2 chunks of 128):
  T1a[t,(b,m)] = x*lr          (DVE f16 2x via x-replicated-pair broadcast)
  S = TRI@T1a + I@C0_bc (+ ONES@T1a_c0 for chunk1)   (PE, f16 rhs, f32 psum)
      C0[t,m] = cumsum_t log eps0  -- host precomputed
  H = exp(S)                   (ACT, 1024-free reads from 2-bank psum tiles)
  WH = H*wsh_bc                (DVE; gpsimd bulk ops poison DVE SBUF access)
  rp/rsn = binary-tree m-reduction in f16 (fp32 final)        (DVE)
  tail: counts via STRI matmul, t-1 shift via SHM/SH2 matmuls, exp/ln
  softplus, masked sum via ones-column matmul.
"""
import os
import sys

import numpy as np

for _p in ("/opt/trn_rl_repo", os.path.expanduser("~/.axon_site/_ro/trn_rl_repo")):
    if os.path.isdir(_p) and _p not in sys.path:
        sys.path.insert(0, _p)
        break

import concourse.bass as bass
import concourse.bacc as bacc
import concourse.mybir as mybir
from concourse.tile import TileContext

B, L, M = 1024, 256, 128
NCORES = 8
BLOC = B // NCORES          # 128 batch rows per core
HALF = L // 2
WB = 16                     # batch rows per wide block
NWB = BLOC // WB            # 8 wide blocks
FB = WB * M                 # 2048 free elems per chunk per wblock
NCHUNK = 2

F32 = mybir.dt.float32
F16 = mybir.dt.float16
AF = mybir.ActivationFunctionType
ALU = mybir.AluOpType
AX = mybir.AxisListType


def build_nc():
    nc = bacc.Bacc("TRN2", target_bir_lowering=False)
    xr2 = nc.dram_tensor("xr2", (128, NCHUNK * WB * NWB * 2), F16,
                         kind="ExternalInput")          # (t, c, b, 2)
    lrt = nc.dram_tensor("lrt", (128, NCHUNK * M), F16, kind="ExternalInput")
    c0t = nc.dram_tensor("c0t", (128, NCHUNK * M), F16, kind="ExternalInput")
    wsht = nc.dram_tensor("wsht", (128, NCHUNK * M), F16, kind="ExternalInput")
    tri3 = nc.dram_tensor("tri3", (128, 3 * 128), F16, kind="ExternalInput")
    shms = nc.dram_tensor("shms", (128, 2 * 128), F16, kind="ExternalInput")
    tvc = nc.dram_tensor("tvc", (128, 4), F32, kind="ExternalInput")
    one0 = nc.dram_tensor("one0", (1, BLOC), F32, kind="ExternalInput")
    cst0 = nc.dram_tensor("cst0", (1, BLOC), F32, kind="ExternalInput")
    y = nc.dram_tensor("y", (1, BLOC), F32, kind="ExternalOutput")

    with TileContext(nc) as tc:
        with (
            tc.tile_pool(name="const", bufs=1) as cpool,
            tc.tile_pool(name="t1p", bufs=3) as t1pool,
            tc.tile_pool(name="hp", bufs=3) as hpool,
            tc.tile_pool(name="whp", bufs=3) as whpool,
            tc.tile_pool(name="s1p", bufs=3) as s1pool,
            tc.tile_pool(name="red", bufs=1) as rpool,
            tc.tile_pool(name="small", bufs=1) as spool,
            tc.tile_pool(name="ps", bufs=2, space="PSUM") as pspool,
            tc.tile_pool(name="psm", bufs=1, space="PSUM") as psmisc,
        ):
            # ---- constants ----
            XR2 = cpool.tile([128, NCHUNK * BLOC * 2], F16, tag="XR2")
            LRT = cpool.tile([128, NCHUNK * M], F16, tag="LRT")
            C0T = cpool.tile([128, NCHUNK * M], F16, tag="C0T")
            WSHT = cpool.tile([128, NCHUNK * M], F16, tag="WSHT")
            TRI3 = cpool.tile([128, 3 * 128], F16, tag="TRI3")
            SHMS = cpool.tile([128, 2 * 128], F16, tag="SHMS")
            TVC = cpool.tile([128, 4], F32, tag="TVC")
            ONE0 = cpool.tile([1, BLOC], F32, tag="ONE0")
            CST0 = cpool.tile([1, BLOC], F32, tag="CST0")
            # critical-path first: first wblocks need xr2 q0 (c0 b<64) and
            # q2 (c1 b<64), then TRI3 for the first matmuls
            nc.sync.dma_start(LRT[:], lrt[:])
            nc.sync.dma_start(XR2[:, 0:32], xr2[:, 0:32])
            nc.sync.dma_start(XR2[:, 256:288], xr2[:, 256:288])
            nc.sync.dma_start(TRI3[:], tri3[:])
            nc.sync.dma_start(XR2[:, 32:128], xr2[:, 32:128])
            nc.sync.dma_start(XR2[:, 288:384], xr2[:, 288:384])
            nc.sync.dma_start(C0T[:], c0t[:])
            for dq in (1, 3):
                qs = slice(dq * 128, (dq + 1) * 128)
                nc.sync.dma_start(XR2[:, qs], xr2[:, qs])
            nc.sync.dma_start(WSHT[:], wsht[:])
            nc.sync.dma_start(TVC[:], tvc[:])
            nc.sync.dma_start(ONE0[:], one0[:])
            nc.sync.dma_start(CST0[:], cst0[:])
            nc.sync.dma_start(SHMS[:], shms[:])
            TRI = TRI3[:, 0:128]
            STRI = TRI3[:, 128:256]
            IDT = TRI3[:, 256:384]
            SHM = SHMS[:, 0:128]
            SH2 = SHMS[:, 128:256]
            ONES16 = cpool.tile([128, 128], F16, tag="ONES16")
            nc.vector.memset(ONES16[:], 1.0)
            ONEC = cpool.tile([128, 1], F32, tag="ONEC")
            nc.vector.memset(ONEC[:], 1.0)
            # (t, c, b, 2) view of XR2
            XRV = XR2[:].rearrange("p (c b two) -> p c b two", c=NCHUNK, two=2)

            # ---- persistent reduction tensors ----
            R64 = rpool.tile([128, NCHUNK, BLOC, 64], F16, tag="R64")
            W64 = rpool.tile([128, NCHUNK, BLOC, 64], F16, tag="W64")
            RP = spool.tile([128, NCHUNK, BLOC], F32, tag="RP")

            # ---- main loop ----
            for wb in range(NWB):
                bsl = slice(wb * WB, (wb + 1) * WB)
                t1 = []
                for c in range(NCHUNK):
                    t = t1pool.tile([128, WB, 64, 2], F16, tag=f"T1_{c}")
                    xbc = (XRV[:, c, bsl, :].unsqueeze(2)
                           .broadcast_to([128, WB, 64, 2]))
                    lbc = (LRT[:, c * M:(c + 1) * M]
                           .rearrange("p (a two) -> p a two", two=2)
                           .unsqueeze(1).broadcast_to([128, WB, 64, 2]))
                    nc.vector.tensor_tensor(t[:], xbc, lbc, ALU.mult)
                    t1.append(t)
                ht = []
                for c in range(NCHUNK):
                    h = hpool.tile([128, WB, M], F16, tag=f"H_{c}")
                    t1f = t1[c][:].rearrange("p b a two -> p (b a two)")
                    t1f0 = t1[0][:].rearrange("p b a two -> p (b a two)")
                    c0bc = (C0T[:, c * M:(c + 1) * M].unsqueeze(1)
                            .broadcast_to([128, 4, M]))
                    P0 = pspool.tile([128, 1024], F32, tag="S")
                    P1 = pspool.tile([128, 1024], F32, tag="S")
                    P = [P0, P1]
                    # 512-wide matmuls (psum bank limit), grouped by weights
                    for q in range(2):
                        for hh in range(2):
                            sl = slice((2 * q + hh) * 512,
                                       (2 * q + hh + 1) * 512)
                            nc.tensor.matmul(P[q][:, hh * 512:(hh + 1) * 512],
                                             TRI, t1f[:, sl],
                                             start=True, stop=False)
                    for q in range(2):
                        for hh in range(2):
                            po = P[q][:, hh * 512:(hh + 1) * 512]
                            nc.tensor.matmul(po, IDT, c0bc,
                                             start=False, stop=(c == 0))
                    if c == 1:
                        for q in range(2):
                            for hh in range(2):
                                sl = slice((2 * q + hh) * 512,
                                           (2 * q + hh + 1) * 512)
                                nc.tensor.matmul(
                                    P[q][:, hh * 512:(hh + 1) * 512],
                                    ONES16, t1f0[:, sl],
                                    start=False, stop=True)
                    for q in range(2):
                        hv = (h[:].rearrange("p b m -> p (b m)")
                              [:, q * 1024:(q + 1) * 1024])
                        nc.scalar.activation(hv, P[q][:], AF.Exp)
                    ht.append(h)
                for c in range(NCHUNK):
                    w = whpool.tile([128, WB, M], F16, tag=f"W_{c}")
                    wbc = (WSHT[:, c * M:(c + 1) * M].unsqueeze(1)
                           .broadcast_to([128, WB, M]))
                    nc.vector.tensor_tensor(w[:], ht[c][:], wbc, ALU.mult)
                    # tree step 1 straight into the persistent pyramids
                    nc.vector.tensor_tensor(R64[:, c, bsl, :],
                                            ht[c][:, :, 0:64],
                                            ht[c][:, :, 64:128], ALU.add)
                    nc.vector.tensor_tensor(W64[:, c, bsl, :],
                                            w[:, :, 0:64],
                                            w[:, :, 64:128], ALU.add)

            # ---- global tree steps + final reduce ----
            # W-side first: its t-1 shift (PE) + psum reduce overlap the
            # rp-side pyramid on DVE
            R32 = rpool.tile([128, NCHUNK, BLOC, 32], F16, tag="R32")
            R16 = rpool.tile([128, NCHUNK, BLOC, 16], F16, tag="R16")
            R8 = rpool.tile([128, NCHUNK, BLOC, 8], F16, tag="R8")
            W8 = rpool.tile([128, NCHUNK, BLOC, 8], F16, tag="W8")
            RSA = spool.tile([128, NCHUNK, BLOC], F32, tag="RSA")
            nc.vector.tensor_tensor(R32[:], W64[:, :, :, 0:32],
                                    W64[:, :, :, 32:64], ALU.add)
            nc.vector.tensor_tensor(R16[:], R32[:, :, :, 0:16],
                                    R32[:, :, :, 16:32], ALU.add)
            nc.vector.tensor_tensor(W8[:], R16[:, :, :, 0:8],
                                    R16[:, :, :, 8:16], ALU.add)
            # shifted-by-one-t version of W8 via SHM/SH2 matmuls (f16)
            for c in range(NCHUNK):
                WS = pspool.tile([128, 1024], F32, tag="S")
                w8v = W8[:, c].rearrange("p b m -> p (b m)")
                w8v0 = W8[:, 0].rearrange("p b m -> p (b m)")
                for hh in range(2):
                    hsl = slice(hh * 512, (hh + 1) * 512)
                    nc.tensor.matmul(WS[:, hsl], SHM, w8v[:, hsl],
                                     start=True, stop=(c == 0))
                    if c == 1:
                        nc.tensor.matmul(WS[:, hsl], SH2, w8v0[:, hsl],
                                         start=False, stop=True)
                nc.vector.tensor_reduce(
                    RSA[:, c, :],
                    WS[:].rearrange("p (b m) -> p b m", m=8),
                    AX.X, ALU.add)
            # RSA[0, c0, :] = rs[0] = s0
            nc.vector.tensor_copy(RSA[0:1, 0, :], CST0[:])
            # rp-side pyramid
            nc.vector.tensor_tensor(R32[:], R64[:, :, :, 0:32],
                                    R64[:, :, :, 32:64], ALU.add)
            nc.vector.tensor_tensor(R16[:], R32[:, :, :, 0:16],
                                    R32[:, :, :, 16:32], ALU.add)
            nc.vector.tensor_tensor(R8[:], R16[:, :, :, 0:8],
                                    R16[:, :, :, 8:16], ALU.add)
            nc.vector.tensor_reduce(RP[:], R8[:], AX.X, ALU.add)

            # ---- tail ----
            # exclusive spin-up counts c1[t,b] via strict-lower-tri matmuls
            X0 = XRV[:, 0, :, 0]                  # (128, BLOC) stride-2 view
            X1 = XRV[:, 1, :, 0]
            C1p = psmisc.tile([128, NCHUNK, BLOC], F32, tag="C1p")
            nc.tensor.matmul(C1p[:, 0, :], STRI, X0, start=True, stop=True)
            nc.tensor.matmul(C1p[:, 1, :], STRI, X1, start=True, stop=False)
            nc.tensor.matmul(C1p[:, 1, :], ONES16, X0, start=False, stop=True)
            # n_other = c1 + x*(t - 2*c1); notmask = n_other < HALF
            NM = spool.tile([128, NCHUNK, BLOC], F32, tag="NM")
            UT = spool.tile([128, NCHUNK, BLOC], F32, tag="UT")
            for c in range(NCHUNK):
                xc = XRV[:, c, :, 0]
                nc.vector.tensor_scalar(UT[:, c, :], C1p[:, c, :], -2.0,
                                        TVC[:, c:c + 1], ALU.mult, ALU.add)
                nc.vector.tensor_tensor(UT[:, c, :], UT[:, c, :], xc, ALU.mult)
                nc.vector.tensor_tensor(UT[:, c, :], UT[:, c, :], C1p[:, c, :],
                                        ALU.add)
                nc.vector.tensor_single_scalar(NM[:, c, :], UT[:, c, :],
                                               float(HALF) - 0.5, ALU.is_lt)
            # term = notmask * (rp - mx - 0.5*log1p(exp(2*(mn-mx))))
            RO = spool.tile([128, NCHUNK, BLOC], F32, tag="RO")
            MX = spool.tile([128, NCHUNK, BLOC], F32, tag="MX")
            MN = spool.tile([128, NCHUNK, BLOC], F32, tag="MN")
            SPt = spool.tile([128, NCHUNK, BLOC], F32, tag="SPt")
            TERM = spool.tile([128, NCHUNK, BLOC], F32, tag="TERM")
            nc.vector.tensor_tensor(RO[:], RSA[:], RP[:], ALU.subtract)
            nc.vector.tensor_tensor(MX[:], RP[:], RO[:], ALU.max)
            nc.vector.tensor_tensor(MN[:], RP[:], RO[:], ALU.min)
            nc.vector.tensor_tensor(MN[:], MN[:], MX[:], ALU.subtract)
            nc.scalar.activation(SPt[:], MN[:], AF.Exp, scale=2.0)
            nc.scalar.activation(SPt[:], SPt[:], AF.Ln, bias=1.0)
            nc.vector.tensor_tensor(MX[:], RP[:], MX[:], ALU.subtract)
            nc.vector.scalar_tensor_tensor(TERM[:], SPt[:], -0.5, MX[:],
                                           ALU.mult, ALU.add)
            nc.vector.tensor_tensor(TERM[:], TERM[:], NM[:], ALU.mult)
            # out[b] = sum_t term, both chunks accumulated in one psum
            YPp = psmisc.tile([1, BLOC], F32, tag="YPp")
            nc.tensor.matmul(YPp[:], ONEC[:], TERM[:, 0, :],
                             start=True, stop=False)
            nc.tensor.matmul(YPp[:], ONEC[:], TERM[:, 1, :],
                             start=False, stop=True)
            YF = spool.tile([1, BLOC], F32, tag="YF")
            nc.scalar.activation(YF[:], YPp[:], AF.Copy)
            nc.sync.dma_start(y[:], YF[:])
    nc.compile()
    return nc


def host_tables(inputs, epsilon):
    x = np.asarray(inputs).astype(np.float32)        # (B, L)
    eps = np.asarray(epsilon).astype(np.float64)     # (2, M, L)
    eps0, eps1 = eps[0], eps[1]
    le0 = np.log(eps0)                               # (M, L)
    le1 = np.log(eps1)
    lr = (le1 - le0)                                 # (M, L)
    c0 = np.cumsum(le0, axis=1)                      # (M, L)
    w = eps0 + eps1
    wsh = np.zeros((M, L))
    wsh[:, :L - 1] = w[:, 1:]
    s0 = np.float32(w[:, 0].sum())

    def chunked_t(a):   # (M, L) -> (128, 2*M): [:, c*M:(c+1)*M] = a[:, c-chunk].T
        return np.concatenate([a[:, c * 128:(c + 1) * 128].T
                               for c in range(NCHUNK)], axis=1)

    lrt = chunked_t(lr).astype(np.float16)
    c0t = chunked_t(c0).astype(np.float16)
    wsht = chunked_t(wsh).astype(np.float16)

    ar = np.arange(128)
    tri = (ar[:, None] <= ar[None, :]).astype(np.float16)
    stri = (ar[:, None] < ar[None, :]).astype(np.float16)
    idt = (ar[:, None] == ar[None, :]).astype(np.float16)
    tri3 = np.ascontiguousarray(np.concatenate([tri, stri, idt], axis=1))
    shm = (ar[:, None] == (ar[None, :] - 1)).astype(np.float16)
    sh2 = ((ar[:, None] == 127) & (ar[None, :] == 0)).astype(np.float16)
    shms = np.ascontiguousarray(np.concatenate([shm, sh2], axis=1))
    tvc = np.zeros((128, 4), np.float32)
    tvc[:, 0] = ar
    tvc[:, 1] = ar + 128.0

    tables = {
        "lrt": lrt, "c0t": c0t, "wsht": wsht, "tri3": tri3, "shms": shms,
        "tvc": tvc,
        "one0": (np.arange(BLOC)[None, :] == 0).astype(np.float32),
        "cst0": np.full((1, BLOC), s0, np.float32),
    }
    # xr2 per core built later: (128, c, b, 2) f16
    xt = x.T                                         # (L, B)
    return tables, xt


_NC_CACHE = {}


def get_nc():
    if "nc" not in _NC_CACHE:
        _NC_CACHE["nc"] = build_nc()
    return _NC_CACHE["nc"]


def make_xr2(xt, k):
    xc = xt[:, k * BLOC:(k + 1) * BLOC]              # (L, BLOC)
    xr2 = np.empty((128, NCHUNK, BLOC, 2), np.float16)
    for c in range(NCHUNK):
        xr2[:, c, :, 0] = xc[c * 128:(c + 1) * 128]
        xr2[:, c, :, 1] = xc[c * 128:(c + 1) * 128]
    return np.ascontiguousarray(xr2.reshape(128, -1))


def kernel(inputs, epsilon):
    from concourse.bass_utils import run_bass_kernel_spmd

    tables, xt = host_tables(inputs, epsilon)
    nc = get_nc()
    in_maps = []
    for k in range(NCORES):
        m = dict(tables)
        m["xr2"] = make_xr2(xt, k)
        in_maps.append(m)
    res = run_bass_kernel_spmd(nc, in_maps, core_ids=list(range(NCORES)))
    out = np.empty((B,), np.float32)
    for k in range(NCORES):
        out[k * BLOC:(k + 1) * BLOC] = np.asarray(res.results[k]["y"]).reshape(-1)
    return out


# revision 9
# speedup vs baseline: 1.0553x; 1.0193x over previous
"""
Trainium2 Bass kernel v2 for nn_ARqGPS (autoregressive qGPS log-amplitude).

Math:
  H[b,m,t]  = prod_{j<=t} eps_sel[b,m,j],  eps_sel = eps[x[b,t],m,t]
  rp[b,t]   = sum_m H[b,m,t]
  rs[b,t]   = sum_m (eps0+eps1)[m,t] * H[b,m,t-1]   (H[.,.,-1] = 1)
  ro = rs - rp;  term = notmask * (rp - mx - 0.5*log1p(exp(2*(mn-mx))))
  out[b] = sum_t term[b,t]

Device (per core, 128 batch rows, t on partitions in 2 chunks of 128):
  T1a[t,(b,m)] = x*lr          (DVE f16 2x via x-replicated-pair broadcast)
  S = TRI@T1a + I@C0_bc (+ ONES@T1a_c0 for chunk1)   (PE, f16 rhs, f32 psum)
      C0[t,m] = cumsum_t log eps0  -- host precomputed
  H = exp(S)                   (ACT, 1024-free reads from 2-bank psum tiles)
  WH = H*wsh_bc                (DVE; gpsimd bulk ops poison DVE SBUF access)
  rp/rsn = binary-tree m-reduction in f16 (fp32 final)        (DVE)
  tail: counts via STRI matmul, t-1 shift via SHM/SH2 matmuls, exp/ln
  softplus, masked sum via ones-column matmul.
"""
import os
import sys

import numpy as np

for _p in ("/opt/trn_rl_repo", os.path.expanduser("~/.axon_site/_ro/trn_rl_repo")):
    if os.path.isdir(_p) and _p not in sys.path:
        sys.path.insert(0, _p)
        break

import concourse.bass as bass
import concourse.bacc as bacc
import concourse.mybir as mybir
from concourse.tile import TileContext

B, L, M = 1024, 256, 128
NCORES = 8
BLOC = B // NCORES          # 128 batch rows per core
HALF = L // 2
WB = 16                     # batch rows per wide block
NWB = BLOC // WB            # 8 wide blocks
FB = WB * M                 # 2048 free elems per chunk per wblock
NCHUNK = 2

F32 = mybir.dt.float32
F16 = mybir.dt.float16
AF = mybir.ActivationFunctionType
ALU = mybir.AluOpType
AX = mybir.AxisListType


def build_nc():
    nc = bacc.Bacc("TRN2", target_bir_lowering=False)
    xr2 = nc.dram_tensor("xr2", (128, NCHUNK * WB * NWB * 2), F16,
                         kind="ExternalInput")          # (t, c, b, 2)
    lrt = nc.dram_tensor("lrt", (128, NCHUNK * M), F16, kind="ExternalInput")
    c0t = nc.dram_tensor("c0t", (128, NCHUNK * M), F16, kind="ExternalInput")
    wsht = nc.dram_tensor("wsht", (128, NCHUNK * M), F16, kind="ExternalInput")
    tri3 = nc.dram_tensor("tri3", (128, 3 * 128), F16, kind="ExternalInput")
    shms = nc.dram_tensor("shms", (128, 2 * 128), F16, kind="ExternalInput")
    tvc = nc.dram_tensor("tvc", (128, 4), F32, kind="ExternalInput")
    one0 = nc.dram_tensor("one0", (1, BLOC), F32, kind="ExternalInput")
    cst0 = nc.dram_tensor("cst0", (1, BLOC), F32, kind="ExternalInput")
    y = nc.dram_tensor("y", (1, BLOC), F32, kind="ExternalOutput")

    with TileContext(nc) as tc:
        with (
            tc.tile_pool(name="const", bufs=1) as cpool,
            tc.tile_pool(name="t1p", bufs=3) as t1pool,
            tc.tile_pool(name="hp", bufs=2) as hpool,
            tc.tile_pool(name="whp", bufs=2) as whpool,
            tc.tile_pool(name="s1p", bufs=3) as s1pool,
            tc.tile_pool(name="red", bufs=1) as rpool,
            tc.tile_pool(name="small", bufs=1) as spool,
            tc.tile_pool(name="ps", bufs=2, space="PSUM") as pspool,
            tc.tile_pool(name="psm", bufs=1, space="PSUM") as psmisc,
        ):
            # ---- constants ----
            XR2 = cpool.tile([128, NCHUNK * BLOC * 2], F16, tag="XR2")
            LRT = cpool.tile([128, NCHUNK * M], F16, tag="LRT")
            C0T = cpool.tile([128, NCHUNK * M], F16, tag="C0T")
            WSHT = cpool.tile([128, NCHUNK * M], F16, tag="WSHT")
            TRI3 = cpool.tile([128, 3 * 128], F16, tag="TRI3")
            SHMS = cpool.tile([128, 2 * 128], F16, tag="SHMS")
            TVC = cpool.tile([128, 4], F32, tag="TVC")
            ONE0 = cpool.tile([1, BLOC], F32, tag="ONE0")
            CST0 = cpool.tile([1, BLOC], F32, tag="CST0")
            # critical-path first: first wblocks need xr2 q0 (c0 b<64) and
            # q2 (c1 b<64), then TRI3 for the first matmuls
            nc.sync.dma_start(LRT[:], lrt[:])
            nc.sync.dma_start(XR2[:, 0:32], xr2[:, 0:32])
            nc.sync.dma_start(XR2[:, 256:288], xr2[:, 256:288])
            nc.sync.dma_start(TRI3[:], tri3[:])
            nc.sync.dma_start(XR2[:, 32:128], xr2[:, 32:128])
            nc.sync.dma_start(XR2[:, 288:384], xr2[:, 288:384])
            nc.sync.dma_start(C0T[:], c0t[:])
            for dq in (1, 3):
                qs = slice(dq * 128, (dq + 1) * 128)
                nc.sync.dma_start(XR2[:, qs], xr2[:, qs])
            nc.sync.dma_start(WSHT[:], wsht[:])
            nc.sync.dma_start(TVC[:], tvc[:])
            nc.sync.dma_start(ONE0[:], one0[:])
            nc.sync.dma_start(CST0[:], cst0[:])
            nc.sync.dma_start(SHMS[:], shms[:])
            TRI = TRI3[:, 0:128]
            STRI = TRI3[:, 128:256]
            IDT = TRI3[:, 256:384]
            SHM = SHMS[:, 0:128]
            SH2 = SHMS[:, 128:256]
            ONES16 = cpool.tile([128, 128], F16, tag="ONES16")
            nc.vector.memset(ONES16[:], 1.0)
            ONEC = cpool.tile([128, 1], F32, tag="ONEC")
            nc.vector.memset(ONEC[:], 1.0)
            # (t, c, b, 2) view of XR2
            XRV = XR2[:].rearrange("p (c b two) -> p c b two", c=NCHUNK, two=2)

            # ---- persistent reduction tensors ----
            R64 = rpool.tile([128, NCHUNK, BLOC, 64], F16, tag="R64")
            W64 = rpool.tile([128, NCHUNK, BLOC, 64], F16, tag="W64")
            RP = spool.tile([128, NCHUNK, BLOC], F32, tag="RP")

            # ---- main loop ----
            hpair = {}
            for wb in range(NWB):
                bsl = slice(wb * WB, (wb + 1) * WB)
                half = wb % 2
                t1 = []
                for c in range(NCHUNK):
                    t = t1pool.tile([128, WB, 64, 2], F16, tag=f"T1_{c}")
                    xbc = (XRV[:, c, bsl, :].unsqueeze(2)
                           .broadcast_to([128, WB, 64, 2]))
                    lbc = (LRT[:, c * M:(c + 1) * M]
                           .rearrange("p (a two) -> p a two", two=2)
                           .unsqueeze(1).broadcast_to([128, WB, 64, 2]))
                    nc.vector.tensor_tensor(t[:], xbc, lbc, ALU.mult)
                    t1.append(t)
                for c in range(NCHUNK):
                    if half == 0:
                        hnew = hpool.tile([128, 2 * WB, M], F16,
                                          tag=f"H_{c}")
                        hpair[c] = hnew
                    h2 = hpair[c]
                    t1f = t1[c][:].rearrange("p b a two -> p (b a two)")
                    t1f0 = t1[0][:].rearrange("p b a two -> p (b a two)")
                    c0bc = (C0T[:, c * M:(c + 1) * M].unsqueeze(1)
                            .broadcast_to([128, 4, M]))
                    P0 = pspool.tile([128, 1024], F32, tag="S")
                    P1 = pspool.tile([128, 1024], F32, tag="S")
                    P = [P0, P1]
                    # 512-wide matmuls (psum bank limit), grouped by weights
                    for q in range(2):
                        for hh in range(2):
                            sl = slice((2 * q + hh) * 512,
                                       (2 * q + hh + 1) * 512)
                            nc.tensor.matmul(P[q][:, hh * 512:(hh + 1) * 512],
                                             TRI, t1f[:, sl],
                                             start=True, stop=False)
                    for q in range(2):
                        for hh in range(2):
                            po = P[q][:, hh * 512:(hh + 1) * 512]
                            nc.tensor.matmul(po, IDT, c0bc,
                                             start=False, stop=(c == 0))
                    if c == 1:
                        for q in range(2):
                            for hh in range(2):
                                sl = slice((2 * q + hh) * 512,
                                           (2 * q + hh + 1) * 512)
                                nc.tensor.matmul(
                                    P[q][:, hh * 512:(hh + 1) * 512],
                                    ONES16, t1f0[:, sl],
                                    start=False, stop=True)
                    for q in range(2):
                        hv = (h2[:].rearrange("p b m -> p (b m)")
                              [:, (2 * half + q) * 1024:
                                  (2 * half + q + 1) * 1024])
                        nc.scalar.activation(hv, P[q][:], AF.Exp)
                if half == 1:
                    bsl2 = slice((wb - 1) * WB, (wb + 1) * WB)
                    for c in range(NCHUNK):
                        h2 = hpair[c]
                        w = whpool.tile([128, 2 * WB, M], F16, tag=f"W_{c}")
                        wbc = (WSHT[:, c * M:(c + 1) * M].unsqueeze(1)
                               .broadcast_to([128, 2 * WB, M]))
                        nc.vector.tensor_tensor(w[:], h2[:], wbc, ALU.mult)
                        # tree step 1 straight into the persistent pyramids
                        nc.vector.tensor_tensor(R64[:, c, bsl2, :],
                                                h2[:, :, 0:64],
                                                h2[:, :, 64:128], ALU.add)
                        nc.vector.tensor_tensor(W64[:, c, bsl2, :],
                                                w[:, :, 0:64],
                                                w[:, :, 64:128], ALU.add)

            # ---- global tree steps + final reduce ----
            # W-side first: its t-1 shift (PE) + psum reduce overlap the
            # rp-side pyramid on DVE
            R32 = rpool.tile([128, NCHUNK, BLOC, 32], F16, tag="R32")
            R16 = rpool.tile([128, NCHUNK, BLOC, 16], F16, tag="R16")
            R8 = rpool.tile([128, NCHUNK, BLOC, 8], F16, tag="R8")
            W8 = rpool.tile([128, NCHUNK, BLOC, 8], F16, tag="W8")
            RSA = spool.tile([128, NCHUNK, BLOC], F32, tag="RSA")
            nc.vector.tensor_tensor(R32[:], W64[:, :, :, 0:32],
                                    W64[:, :, :, 32:64], ALU.add)
            nc.vector.tensor_tensor(R16[:], R32[:, :, :, 0:16],
                                    R32[:, :, :, 16:32], ALU.add)
            nc.vector.tensor_tensor(W8[:], R16[:, :, :, 0:8],
                                    R16[:, :, :, 8:16], ALU.add)
            # shifted-by-one-t version of W8 via SHM/SH2 matmuls (f16)
            for c in range(NCHUNK):
                WS = pspool.tile([128, 1024], F32, tag="S")
                w8v = W8[:, c].rearrange("p b m -> p (b m)")
                w8v0 = W8[:, 0].rearrange("p b m -> p (b m)")
                for hh in range(2):
                    hsl = slice(hh * 512, (hh + 1) * 512)
                    nc.tensor.matmul(WS[:, hsl], SHM, w8v[:, hsl],
                                     start=True, stop=(c == 0))
                    if c == 1:
                        nc.tensor.matmul(WS[:, hsl], SH2, w8v0[:, hsl],
                                         start=False, stop=True)
                nc.vector.tensor_reduce(
                    RSA[:, c, :],
                    WS[:].rearrange("p (b m) -> p b m", m=8),
                    AX.X, ALU.add)
            # RSA[0, c0, :] = rs[0] = s0
            nc.vector.tensor_copy(RSA[0:1, 0, :], CST0[:])
            # rp-side pyramid
            nc.vector.tensor_tensor(R32[:], R64[:, :, :, 0:32],
                                    R64[:, :, :, 32:64], ALU.add)
            nc.vector.tensor_tensor(R16[:], R32[:, :, :, 0:16],
                                    R32[:, :, :, 16:32], ALU.add)
            nc.vector.tensor_tensor(R8[:], R16[:, :, :, 0:8],
                                    R16[:, :, :, 8:16], ALU.add)
            nc.vector.tensor_reduce(RP[:], R8[:], AX.X, ALU.add)

            # ---- tail ----
            # exclusive spin-up counts c1[t,b] via strict-lower-tri matmuls
            X0 = XRV[:, 0, :, 0]                  # (128, BLOC) stride-2 view
            X1 = XRV[:, 1, :, 0]
            C1p = psmisc.tile([128, NCHUNK, BLOC], F32, tag="C1p")
            nc.tensor.matmul(C1p[:, 0, :], STRI, X0, start=True, stop=True)
            nc.tensor.matmul(C1p[:, 1, :], STRI, X1, start=True, stop=False)
            nc.tensor.matmul(C1p[:, 1, :], ONES16, X0, start=False, stop=True)
            # n_other = c1 + x*(t - 2*c1); notmask = n_other < HALF
            NM = spool.tile([128, NCHUNK, BLOC], F32, tag="NM")
            UT = spool.tile([128, NCHUNK, BLOC], F32, tag="UT")
            for c in range(NCHUNK):
                xc = XRV[:, c, :, 0]
                nc.vector.tensor_scalar(UT[:, c, :], C1p[:, c, :], -2.0,
                                        TVC[:, c:c + 1], ALU.mult, ALU.add)
                nc.vector.tensor_tensor(UT[:, c, :], UT[:, c, :], xc, ALU.mult)
                nc.vector.tensor_tensor(UT[:, c, :], UT[:, c, :], C1p[:, c, :],
                                        ALU.add)
                nc.vector.tensor_single_scalar(NM[:, c, :], UT[:, c, :],
                                               float(HALF) - 0.5, ALU.is_lt)
            # term = notmask * (rp - mx - 0.5*log1p(exp(2*(mn-mx))))
            RO = spool.tile([128, NCHUNK, BLOC], F32, tag="RO")
            MX = spool.tile([128, NCHUNK, BLOC], F32, tag="MX")
            MN = spool.tile([128, NCHUNK, BLOC], F32, tag="MN")
            SPt = spool.tile([128, NCHUNK, BLOC], F32, tag="SPt")
            TERM = spool.tile([128, NCHUNK, BLOC], F32, tag="TERM")
            nc.vector.tensor_tensor(RO[:], RSA[:], RP[:], ALU.subtract)
            nc.vector.tensor_tensor(MX[:], RP[:], RO[:], ALU.max)
            nc.vector.tensor_tensor(MN[:], RP[:], RO[:], ALU.min)
            nc.vector.tensor_tensor(MN[:], MN[:], MX[:], ALU.subtract)
            nc.scalar.activation(SPt[:], MN[:], AF.Exp, scale=2.0)
            nc.scalar.activation(SPt[:], SPt[:], AF.Ln, bias=1.0)
            nc.vector.tensor_tensor(MX[:], RP[:], MX[:], ALU.subtract)
            nc.vector.scalar_tensor_tensor(TERM[:], SPt[:], -0.5, MX[:],
                                           ALU.mult, ALU.add)
            nc.vector.tensor_tensor(TERM[:], TERM[:], NM[:], ALU.mult)
            # out[b] = sum_t term, both chunks accumulated in one psum
            YPp = psmisc.tile([1, BLOC], F32, tag="YPp")
            nc.tensor.matmul(YPp[:], ONEC[:], TERM[:, 0, :],
                             start=True, stop=False)
            nc.tensor.matmul(YPp[:], ONEC[:], TERM[:, 1, :],
                             start=False, stop=True)
            YF = spool.tile([1, BLOC], F32, tag="YF")
            nc.scalar.activation(YF[:], YPp[:], AF.Copy)
            nc.sync.dma_start(y[:], YF[:])
    nc.compile()
    return nc


def host_tables(inputs, epsilon):
    x = np.asarray(inputs).astype(np.float32)        # (B, L)
    eps = np.asarray(epsilon).astype(np.float64)     # (2, M, L)
    eps0, eps1 = eps[0], eps[1]
    le0 = np.log(eps0)                               # (M, L)
    le1 = np.log(eps1)
    lr = (le1 - le0)                                 # (M, L)
    c0 = np.cumsum(le0, axis=1)                      # (M, L)
    w = eps0 + eps1
    wsh = np.zeros((M, L))
    wsh[:, :L - 1] = w[:, 1:]
    s0 = np.float32(w[:, 0].sum())

    def chunked_t(a):   # (M, L) -> (128, 2*M): [:, c*M:(c+1)*M] = a[:, c-chunk].T
        return np.concatenate([a[:, c * 128:(c + 1) * 128].T
                               for c in range(NCHUNK)], axis=1)

    lrt = chunked_t(lr).astype(np.float16)
    c0t = chunked_t(c0).astype(np.float16)
    wsht = chunked_t(wsh).astype(np.float16)

    ar = np.arange(128)
    tri = (ar[:, None] <= ar[None, :]).astype(np.float16)
    stri = (ar[:, None] < ar[None, :]).astype(np.float16)
    idt = (ar[:, None] == ar[None, :]).astype(np.float16)
    tri3 = np.ascontiguousarray(np.concatenate([tri, stri, idt], axis=1))
    shm = (ar[:, None] == (ar[None, :] - 1)).astype(np.float16)
    sh2 = ((ar[:, None] == 127) & (ar[None, :] == 0)).astype(np.float16)
    shms = np.ascontiguousarray(np.concatenate([shm, sh2], axis=1))
    tvc = np.zeros((128, 4), np.float32)
    tvc[:, 0] = ar
    tvc[:, 1] = ar + 128.0

    tables = {
        "lrt": lrt, "c0t": c0t, "wsht": wsht, "tri3": tri3, "shms": shms,
        "tvc": tvc,
        "one0": (np.arange(BLOC)[None, :] == 0).astype(np.float32),
        "cst0": np.full((1, BLOC), s0, np.float32),
    }
    # xr2 per core built later: (128, c, b, 2) f16
    xt = x.T                                         # (L, B)
    return tables, xt


_NC_CACHE = {}


def get_nc():
    if "nc" not in _NC_CACHE:
        _NC_CACHE["nc"] = build_nc()
    return _NC_CACHE["nc"]


def make_xr2(xt, k):
    xc = xt[:, k * BLOC:(k + 1) * BLOC]              # (L, BLOC)
    xr2 = np.empty((128, NCHUNK, BLOC, 2), np.float16)
    for c in range(NCHUNK):
        xr2[:, c, :, 0] = xc[c * 128:(c + 1) * 128]
        xr2[:, c, :, 1] = xc[c * 128:(c + 1) * 128]
    return np.ascontiguousarray(xr2.reshape(128, -1))


def kernel(inputs, epsilon):
    from concourse.bass_utils import run_bass_kernel_spmd

    tables, xt = host_tables(inputs, epsilon)
    nc = get_nc()
    in_maps = []
    for k in range(NCORES):
        m = dict(tables)
        m["xr2"] = make_xr2(xt, k)
        in_maps.append(m)
    res = run_bass_kernel_spmd(nc, in_maps, core_ids=list(range(NCORES)))
    out = np.empty((B,), np.float32)
    for k in range(NCORES):
        out[k * BLOC:(k + 1) * BLOC] = np.asarray(res.results[k]["y"]).reshape(-1)
    return out
